# revision 2
# baseline (speedup 1.0000x reference)
"""HGT layer on 8 Trainium2 NeuronCores — fused block-aligned design (v3).

Sharding: data-parallel over destination-paper partitions (8 x 25000).
Host: bin-pack each core's dsts into blocks (<=128 dsts, cites edges
<= 128*tc, writes <= 128*tw, tc/tw in {1,2}); block-type composition
harmonized across cores (one SPMD program).  Per edge tile the host
gathers + transposes src features and precomputes the two one-hot
matrices (fp8: exact 0/1) used as matmul stationary operands:
st[e,dst] for dst-aggregation, st2[dst,e] for the q-gather.

Device pass 1, per group of 4 blocks: own hp -> resident hsT; q per
block; then per block-pair half and per relation a "quad" of 2-4 edge
tiles: batched hp/relu, per-tile kv matmul, q-gather matmul
(lhsT=st2), qe copied out on ACT, alpha = DVE prod + per-head reduce,
exp (ACT), v*w (DVE), aggregation matmul (lhsT=st, 132 cols: 128
numerator + 4 denominator); per-half softmax-normalize into resident
op_sum (add on GpSimd).  Pass 2: PE transpose + Gelu + batched final
projections (stationary weights) -> transposed output; host
inverse-permutes.
"""
import numpy as np
import ml_dtypes

import concourse.bass as bass
import concourse.bacc as bacc
import concourse.mybir as mybir
import concourse.tile as tile
from concourse.bass_utils import run_bass_kernel_spmd
from concourse.masks import make_identity

BF = ml_dtypes.bfloat16
F8 = ml_dtypes.float8_e4m3
F32 = mybir.dt.float32
BF16 = mybir.dt.bfloat16
FP8 = mybir.dt.float8e4
AF = mybir.ActivationFunctionType
ALU = mybir.AluOpType

HID, H, D = 128, 4, 32
OUT = 16
SCALE = float(1.0 / np.sqrt(D))
BG = 4          # blocks per group

_BUILD_CACHE = {}


# ---------------------------------------------------------------- host side

def _fold_weights(inp):
    def bd(rel):
        m = np.zeros((HID, HID), np.float32)
        for h in range(H):
            m[h * D:(h + 1) * D, h * D:(h + 1) * D] = rel[h]
        return m

    f = {}
    w1p = np.asarray(inp["lin_paper_w"], np.float32)      # [FP, HID]
    w1a = np.asarray(inp["lin_author_w"], np.float32)     # [FA, HID]
    fp = w1p.shape[0]
    ncp = fp // 128
    f["b1p"] = np.asarray(inp["lin_paper_b"], np.float32).reshape(HID, 1)
    f["b1a"] = np.asarray(inp["lin_author_b"], np.float32).reshape(HID, 1)
    f["w1p"] = np.ascontiguousarray(
        w1p.reshape(ncp, 128, HID).transpose(1, 0, 2).reshape(128, ncp * HID)
    ).astype(BF)
    f["w1a"] = w1a.astype(BF)

    ks = {}
    bks = {}
    for rel, t, e in (("c", "paper", "cites"), ("w", "author", "writes")):
        bda = bd(np.asarray(inp[f"a_rel_{e}"], np.float32))
        bdm = bd(np.asarray(inp[f"m_rel_{e}"], np.float32))
        scl = np.repeat(np.asarray(inp[f"p_rel_{e}"], np.float32) * SCALE, D)
        wk = (np.asarray(inp[f"k_w_{t}"], np.float32) @ bda) * scl[None, :]
        bk = (np.asarray(inp[f"k_b_{t}"], np.float32) @ bda) * scl
        wv = np.asarray(inp[f"v_w_{t}"], np.float32) @ bdm
        bv = np.asarray(inp[f"v_b_{t}"], np.float32) @ bdm
        f[f"wkv_{rel}"] = np.concatenate([wk, wv], axis=1).astype(BF)  # [HID,256]
        ks[rel] = (wk, wv, bv)
        bks[rel] = bk

    # q with optional bk-augmentation columns:
    # alpha += q_e . bk  per relation/head -> extra q columns = q @ BKH
    wq = np.asarray(inp["q_w_paper"], np.float32)
    bq = np.asarray(inp["q_b_paper"], np.float32)
    has_bk = bool(np.any(bks["c"]) or np.any(bks["w"]))
    if has_bk:
        BKH = np.zeros((HID, 8), np.float32)
        for ri, rel in enumerate(("c", "w")):
            for h in range(H):
                BKH[h * D:(h + 1) * D, 4 * ri + h] = bks[rel][h * D:(h + 1) * D]
        f["wq"] = np.concatenate([wq, wq @ BKH], axis=1).astype(BF)  # [128,136]
        f["bq"] = np.concatenate([bq, bq @ BKH]).reshape(1, 136).astype(BF)
    else:
        f["wq"] = wq.astype(BF)
        f["bq"] = bq.reshape(1, HID).astype(BF)
    f["qw_cols"] = 136 if has_bk else 128
    f["has_bq"] = bool(np.any(bq))

    # gelu input bias = bv_c + bv_w (per hid, applied on transposed op)
    f["bgelu"] = (ks["c"][2] + ks["w"][2]).reshape(HID, 1).astype(np.float32)

    beta = 1.0 / (1.0 + np.exp(-float(np.asarray(inp["skip_paper"]))))
    aw = np.asarray(inp["a_w_paper"], np.float32)
    ab = np.asarray(inp["a_b_paper"], np.float32)
    lo_w = np.asarray(inp["lin_out_w"], np.float32)
    lo_b = np.asarray(inp["lin_out_b"], np.float32)
    wg = beta * (aw @ lo_w)
    wh = (1.0 - beta) * lo_w
    f["wgh"] = np.concatenate([wg, wh], axis=1).astype(BF)  # [HID, 32]
    f["bfin"] = (beta * (ab @ lo_w) + lo_b).reshape(OUT, 1).astype(np.float32)
    f["has_bfin"] = bool(np.any(f["bfin"]))
    f["flags"] = (f["qw_cols"], f["has_bq"], f["has_bfin"])
    return f


def _ceil128(x):
    return max(1, (int(x) + 127) // 128)


def _pack_core(cdeg, wdeg):
    """Two-class best-fit packing. Returns list of (members, tc, tw)."""
    sh = len(cdeg)
    td = cdeg + wdeg

    def pack_class(idx, capd, capc, capw, K=12):
        binc = np.zeros(K, np.int64)
        binw = np.zeros(K, np.int64)
        bind = np.zeros(K, np.int64)
        members = [[] for _ in range(K)]
        closed = []
        order = idx[np.argsort(-(cdeg[idx] + wdeg[idx]), kind="stable")]
        for dst in order:
            dc, dw = cdeg[dst], wdeg[dst]
            nc_, nw_, nd_ = binc + dc, binw + dw, bind + 1
            ok = (nd_ <= capd) & (nc_ <= capc) & (nw_ <= capw)
            if not ok.any():
                j = int(np.argmax(binc + binw))
                closed.append((members[j], binc[j], binw[j]))
                binc[j] = binw[j] = bind[j] = 0
                members[j] = []
                nc_, nw_, nd_ = binc + dc, binw + dw, bind + 1
                ok = (nd_ <= capd) & (nc_ <= capc) & (nw_ <= capw)
            score = (capc - nc_) + (capw - nw_)
            score[~ok] = 1 << 30
            j = int(np.argmin(score))
            binc[j] += dc
            binw[j] += dw
            bind[j] += 1
            members[j].append(dst)
        for j in range(K):
            if bind[j]:
                closed.append((members[j], binc[j], binw[j]))
        return closed

    k = int(sh * 0.56)
    low = np.argsort(td, kind="stable")
    closed = pack_class(low[:k], 128, 128, 128) + \
        pack_class(low[k:], 128, 256, 256)
    return [(mem, _ceil128(c), _ceil128(w)) for mem, c, w in closed]


def _prep_all(inputs):
    fold = _fold_weights(inputs)
    xp = np.asarray(inputs["x_paper"], np.float32).astype(BF)
    xa = np.asarray(inputs["x_author"], np.float32).astype(BF)
    NPP, FP = xp.shape
    NA, FA = xa.shape
    ncp, nca = FP // 128, FA // 128
    n_cores = 8
    sh = NPP // n_cores
    xp_aug = np.vstack([xp, np.zeros((1, FP), BF)])
    xa_aug = np.vstack([xa, np.zeros((1, FA), BF)])
    zp, za = NPP, NA

    cs = np.asarray(inputs["cites_src"]).astype(np.int64)
    cd = np.asarray(inputs["cites_dst"]).astype(np.int64)
    ws = np.asarray(inputs["writes_src"]).astype(np.int64)
    wd = np.asarray(inputs["writes_dst"]).astype(np.int64)

    # pack each core, classify block types, harmonize counts across cores
    packs = []
    tcounts = np.zeros((n_cores, 2, 2), np.int64)
    for m in range(n_cores):
        lo, hi = m * sh, (m + 1) * sh
        cdeg = np.bincount(cd[(cd >= lo) & (cd < hi)] - lo, minlength=sh)
        wdeg = np.bincount(wd[(wd >= lo) & (wd < hi)] - lo, minlength=sh)
        blocks = _pack_core(cdeg, wdeg)
        packs.append(blocks)
        for _, tc, tw in blocks:
            tcounts[m, tc - 1, tw - 1] += 1
    ntype = tcounts.max(axis=0)
    NB = int(ntype.sum())
    if NB % BG:
        ntype[0, 0] += BG - NB % BG
        NB = int(ntype.sum())
    type_order = [(1, 1), (1, 2), (2, 1), (2, 2)]
    slot_types = []
    for tc, tw in type_order:
        slot_types += [(tc, tw)] * int(ntype[tc - 1, tw - 1])
    tc_list = [t[0] for t in slot_types]
    tw_list = [t[1] for t in slot_types]
    NTC, NTW = sum(tc_list), sum(tw_list)

    cfg = dict(n_cores=n_cores, sh=sh, NB=NB, NG=NB // BG,
               NTC=NTC, NTW=NTW, ncp=ncp, nca=nca,
               tc=tuple(tc_list), tw=tuple(tw_list),
               flags=fold["flags"])

    in_maps = []
    shared = {k: fold[k] for k in (
        "w1p", "w1a", "b1p", "b1a", "wkv_c", "wkv_w", "wq", "bq",
        "wgh", "bfin", "bgelu")}
    for m in range(n_cores):
        lo, hi = m * sh, (m + 1) * sh
        blocks = packs[m]
        byt = {t: [] for t in type_order}
        for mem, tc, tw in blocks:
            byt[(tc, tw)].append(mem)
        slot_members = []
        for tc, tw in type_order:
            lst = byt[(tc, tw)]
            n = int(ntype[tc - 1, tw - 1])
            assert len(lst) <= n
            slot_members += lst + [[] for _ in range(n - len(lst))]

        perm = np.full(NB * 128, -1, np.int64)
        own_idx = np.full(NB * 128, zp, np.int64)
        dst2slot = np.full(sh, -1, np.int64)
        for b, mem in enumerate(slot_members):
            for j, dloc in enumerate(mem):
                perm[b * 128 + j] = dloc
                own_idx[b * 128 + j] = lo + dloc
                dst2slot[dloc] = b * 128 + j
        assert (dst2slot >= 0).all()
        xg = xp_aug[own_idx]
        xg_own = np.ascontiguousarray(
            xg.reshape(NB // BG, BG, 128, ncp, 128)
            .transpose(0, 4, 1, 3, 2)).reshape(NB // BG, 128, BG * ncp * 128)

        def prep_rel(src, dst, x_aug, zrow, nck, tlist):
            sel = (dst >= lo) & (dst < hi)
            s, dl = src[sel], dst[sel] - lo
            slot = dst2slot[dl]
            b_of = slot // 128
            order = np.argsort(slot, kind="stable")
            s, slot, b_of = s[order], slot[order], b_of[order]
            NT = sum(tlist)
            pad_src = np.full(NT * 128, zrow, np.int64)
            dstl = np.full(NT * 128, 255, np.int64)  # pad sentinel
            toff = np.cumsum([0] + list(tlist))
            bstart = np.searchsorted(b_of, np.arange(NB))
            bend = np.searchsorted(b_of, np.arange(NB), side="right")
            for b in range(NB):
                a, e = bstart[b], bend[b]
                n = e - a
                cap = tlist[b] * 128
                assert n <= cap, (b, n, cap)
                o = toff[b] * 128
                pad_src[o:o + n] = s[a:e]
                dstl[o:o + n] = slot[a:e] % 128
            xg = x_aug[pad_src]
            xT = np.ascontiguousarray(
                xg.reshape(NT, 128, nck, 128).transpose(0, 3, 2, 1)
            ).reshape(NT, 128, nck * 128)
            # one-hot matrices (fp8, exact 0/1)
            dstl2 = dstl.reshape(NT, 128)
            eye = np.eye(128, dtype=np.float32)
            z = np.zeros((1, 128), np.float32)
            lut = np.vstack([eye, z])  # index 128 -> zero row (pad)
            st = lut[np.where(dstl2 < 128, dstl2, 128)]   # [NT,128e,128d]
            st2 = st.transpose(0, 2, 1)
            stp = np.concatenate([st, st2], axis=2)  # [NT,128,256]
            return (xT, np.ascontiguousarray(stp).astype(F8))

        xg_c, stp_c = prep_rel(cs, cd, xp_aug, zp, ncp, tc_list)
        xg_w, stp_w = prep_rel(ws, wd, xa_aug, za, nca, tw_list)

        core = dict(shared)
        core.update(xg_own=xg_own.astype(BF), xg_c=xg_c.astype(BF),
                    xg_w=xg_w.astype(BF), stp_c=stp_c, stp_w=stp_w)
        in_maps.append(core)
        packs[m] = perm
    return cfg, in_maps, packs


# -------------------------------------------------------------- device side

def build_program(cfg, debug=False):
    NB, NG = cfg["NB"], cfg["NG"]
    NTC, NTW = cfg["NTC"], cfg["NTW"]
    ncp, nca = cfg["ncp"], cfg["nca"]
    tc_list, tw_list = cfg["tc"], cfg["tw"]
    QW, has_bq, has_bfin = cfg["flags"]

    nc = bacc.Bacc()
    P = {}

    def par(name, shape, dt):
        P[name] = nc.declare_dram_parameter(name, list(shape), dt, isOutput=False)
        return P[name]

    par("xg_own", [NG, 128, BG * ncp * 128], BF16)
    par("xg_c", [NTC, 128, ncp * 128], BF16)
    par("xg_w", [NTW, 128, nca * 128], BF16)
    par("stp_c", [NTC, 128, 256], FP8)
    par("stp_w", [NTW, 128, 256], FP8)
    par("w1p", [128, ncp * 128], BF16)
    par("w1a", [128, nca * 128], BF16)
    par("b1p", [128, 1], F32)
    par("b1a", [128, 1], F32)
    par("wkv_c", [128, 256], BF16)
    par("wkv_w", [128, 256], BF16)
    par("wq", [128, QW], BF16)
    par("bq", [1, QW], BF16)
    par("wgh", [128, 2 * OUT], BF16)
    par("bfin", [OUT, 1], F32)
    par("bgelu", [128, 1], F32)
    out_t = nc.declare_dram_parameter("out", [OUT, NB * 128], F32, isOutput=True)
    dbg = {}
    if debug:
        for nm, shape, dt in (
                ("hs", [128, NB * 128], BF16), ("op", [128, NB * 128], BF16),
                ("qsb", [128, BG * QW], BF16), ("he", [128, 512], BF16),
                ("qesb", [128, 4 * QW], BF16), ("prod", [128, 512], BF16),
                ("alpha", [128, 16], F32), ("expw", [128, 16], BF16),
                ("rhs", [128, 528], BF16), ("tmpc", [128, 256], BF16),
                ("tmpw", [128, 256], BF16), ("rhsw", [128, 528], BF16),
                ("hew", [128, 512], BF16), ("aggd", [128, 528], F32),
                ("rcpd", [128, 16], F32)):
            dbg[nm] = nc.declare_dram_parameter(
                f"dbg_{nm}", shape, dt, isOutput=True)

    coff = np.cumsum([0] + list(tc_list))
    woff = np.cumsum([0] + list(tw_list))

    with tile.TileContext(nc) as tc_:
        with tc_.tile_pool(name="const", bufs=1) as cp:
            def cload(name, shape2d, dt):
                t = cp.tile(list(shape2d), dt, tag=f"c_{name}", name=f"c_{name}")
                nc.sync.dma_start(out=t[:], in_=P[name][:])
                return t
            t_w1p = cload("w1p", [128, ncp * 128], BF16)
            t_w1a = cload("w1a", [128, nca * 128], BF16)
            t_b1p = cload("b1p", [128, 1], F32)
            t_b1a = cload("b1a", [128, 1], F32)
            t_wkv_c = cload("wkv_c", [128, 256], BF16)
            t_wkv_w = cload("wkv_w", [128, 256], BF16)
            t_wq = cload("wq", [128, QW], BF16)
            t_wgh = cload("wgh", [128, 2 * OUT], BF16)
            t_bgelu = cload("bgelu", [128, 1], F32)
            t_ident = cp.tile([128, 128], BF16)
            make_identity(nc, t_ident[:])
            if has_bq:
                t_bq = cload("bq", [1, QW], BF16)
                t_ones = cp.tile([1, 128], BF16)
                nc.vector.memset(t_ones[:], 1.0)
            if has_bfin:
                t_bfin = cload("bfin", [OUT, 1], F32)

            hs_gt = [cp.tile([128, BG * 128], BF16, tag=f"hs{g}",
                             name=f"hs{g}") for g in range(NG)]
            op_gt = [cp.tile([128, BG * 128], BF16, tag=f"op{g}",
                             name=f"op{g}") for g in range(NG)]

            # ---------------- pass 1 ----------------
            with tc_.tile_pool(name="sb", bufs=4) as sb, \
                 tc_.tile_pool(name="sb_st", bufs=3) as sbst, \
                 tc_.tile_pool(name="sb_x", bufs=3) as sbk, \
                 tc_.tile_pool(name="ps_hpq", bufs=2, space="PSUM") as ps_hpq, \
                 tc_.tile_pool(name="ps_kvq", bufs=2, space="PSUM") as ps_kv, \
                 tc_.tile_pool(name="ps_agg", bufs=1, space="PSUM") as ps_aggq:

                for g in range(NG):
                    b0 = g * BG
                    # ---- own stage ----
                    xo = sbk.tile([128, BG * ncp * 128], BF16, tag="xo")
                    nc.sync.dma_start(out=xo[:], in_=P["xg_own"][g])
                    xo_v = xo[:].rearrange("p (b c e) -> p b c e", b=BG, c=ncp)
                    hp_ps = ps_hpq.tile([128, BG * 128], F32, tag="hpq", name="hp_ps")
                    for c in range(ncp):
                        nc.tensor.matmul(
                            out=hp_ps[:], lhsT=t_w1p[:, c * 128:(c + 1) * 128],
                            rhs=xo_v[:, :, c, :],
                            start=(c == 0), stop=(c == ncp - 1))
                    hs_g = hs_gt[g]
                    nc.scalar.activation(out=hs_g[:], in_=hp_ps[:], func=AF.Relu,
                                         bias=t_b1p[:, :1], scale=1.0)
                    q_ps = ps_kv.tile([128, 1024], F32, tag="kv", name="q_ps")
                    for b in range(BG):
                        nc.tensor.matmul(
                            out=q_ps[:, b * QW:(b + 1) * QW],
                            lhsT=hs_g[:, b * 128:(b + 1) * 128], rhs=t_wq[:],
                            start=True, stop=not has_bq)
                        if has_bq:
                            nc.tensor.matmul(
                                out=q_ps[:, b * QW:(b + 1) * QW],
                                lhsT=t_ones[:1, :], rhs=t_bq[:1, :],
                                start=False, stop=True)
                    q_sb = sb.tile([128, BG * QW], BF16, tag="qsb")
                    nc.scalar.activation(out=q_sb[:], in_=q_ps[:, :BG * QW],
                                         func=AF.Copy)
                    if debug and g == 0:
                        nc.sync.dma_start(out=dbg["qsb"][:], in_=q_sb[:])

                    # ---- edge halves ----
                    for hf in range(2):
                        bl0 = b0 + 2 * hf
                        # one PSUM bank (512 f32) per block: c at +0, w at
                        # +132, dens inside each 132 region at +128 (a
                        # matmul accumulation region must not cross banks)
                        agg = ps_aggq.tile([128, 1024], F32, tag="agg", name="agg")
                        for rel, t_w1x, nck, t_b1x, t_wkvx, xg_p, stp, off \
                            in (("c", t_w1p, ncp, t_b1p, t_wkv_c, P["xg_c"],
                                 P["stp_c"], coff),
                                ("w", t_w1a, nca, t_b1a, t_wkv_w, P["xg_w"],
                                 P["stp_w"], woff)):
                            tlist = tc_list if rel == "c" else tw_list
                            n0, n1 = tlist[bl0], tlist[bl0 + 1]
                            nt = n0 + n1
                            t0 = off[bl0]
                            # tile meta: (local idx, block 0/1, first, last)
                            tmeta = [(j, 0, j == 0, j == n0 - 1)
                                     for j in range(n0)] + \
                                    [(n0 + j, 1, j == 0, j == n1 - 1)
                                     for j in range(n1)]
                            # loads
                            xc = sbk.tile([128, 4 * nck * 128], BF16,
                                          tag=f"x{rel}")
                            nc.sync.dma_start(
                                out=xc[:, :nt * nck * 128].rearrange(
                                    "p (t e) -> p t e", t=nt),
                                in_=xg_p[t0:t0 + nt].rearrange(
                                    "t p e -> p t e"))
                            stt = sbst.tile([128, 4 * 256], FP8, tag="stt")
                            nc.sync.dma_start(
                                out=stt[:, :nt * 256].rearrange(
                                    "p (t e) -> p t e", t=nt),
                                in_=stp[t0:t0 + nt].rearrange("t p e -> p t e"))
                            # hp + relu
                            xc_v = xc[:].rearrange(
                                "p (t c e) -> p t c e", t=4, c=nck)
                            hp2 = ps_hpq.tile([128, BG * 128], F32, tag="hpq", name="hp2")
                            for c in range(nck):
                                nc.tensor.matmul(
                                    out=hp2[:, :nt * 128],
                                    lhsT=t_w1x[:, c * 128:(c + 1) * 128],
                                    rhs=xc_v[:, :nt, c, :],
                                    start=(c == 0), stop=(c == nck - 1))
                            he = sb.tile([128, 4 * 128], BF16, tag=f"he{rel}")
                            nc.scalar.activation(
                                out=he[:, :nt * 128], in_=hp2[:, :nt * 128],
                                func=AF.Relu, bias=t_b1x[:, :1], scale=1.0)
                            # kv + q-gather matmuls
                            kv = ps_kv.tile([128, 4 * 256], F32, tag="kv", name="kv")
                            qe = ps_hpq.tile([128, BG * 128], F32, tag="hpq", name="qe")
                            for j, bl, fi, la in tmeta:
                                nc.tensor.matmul(
                                    out=kv[:, j * 256:(j + 1) * 256],
                                    lhsT=he[:, j * 128:(j + 1) * 128],
                                    rhs=t_wkvx[:], start=True, stop=True)
                                nc.tensor.matmul(
                                    out=qe[:, j * QW:(j + 1) * QW],
                                    lhsT=stt[:, j * 256 + 128:(j + 1) * 256],
                                    rhs=q_sb[:, (2 * hf + bl) * QW:
                                             (2 * hf + bl + 1) * QW],
                                    start=True, stop=True)
                            # alpha path (quad-sized ops)
                            qe_sb = sb.tile([128, 4 * QW], BF16, tag="qesb")
                            nc.scalar.activation(
                                out=qe_sb[:, :nt * QW], in_=qe[:, :nt * QW],
                                func=AF.Copy)
                            prod = sb.tile([128, 4 * 128], BF16, tag="prod")
                            nc.vector.tensor_tensor(
                                out=prod[:, :nt * 128].rearrange(
                                    "p (t x) -> p t x", t=nt),
                                in0=kv[:].rearrange(
                                    "p (t x) -> p t x", t=4)[:, :nt, 0:128],
                                in1=qe_sb[:].rearrange(
                                    "p (t x) -> p t x", t=4)[:, :nt, 0:128],
                                op=ALU.mult)
                            alpha = sb.tile([128, 16], F32, tag="alpha")
                            nc.vector.tensor_reduce(
                                out=alpha[:, :nt * H].rearrange(
                                    "p (t h) -> p t h", h=H),
                                in_=prod[:, :nt * 128].rearrange(
                                    "p (t h x) -> p t h x", h=H, x=D),
                                axis=mybir.AxisListType.X, op=ALU.add)
                            if QW > 128:
                                ri = 0 if rel == "c" else 1
                                nc.vector.tensor_tensor(
                                    out=alpha[:, :nt * H],
                                    in0=alpha[:, :nt * H],
                                    in1=qe_sb[:].rearrange(
                                        "p (t x) -> p t x", t=4
                                    )[:, :nt, 128 + 4 * ri:128 + 4 * ri + H
                                      ].rearrange("p t h -> p (t h)"),
                                    op=ALU.add)
                            expw = sb.tile([128, 16], BF16, tag="expw")
                            nc.scalar.activation(out=expw[:, :nt * H],
                                                 in_=alpha[:, :nt * H],
                                                 func=AF.Exp)
                            rhs_t = sb.tile([128, 4 * 132], BF16, tag="rhs")
                            rv = rhs_t[:].rearrange("p (t x) -> p t x", t=4)
                            nc.vector.tensor_tensor(
                                out=rv[:, :nt, 0:128].rearrange(
                                    "p t (h x) -> p t h x", h=H),
                                in0=kv[:].rearrange(
                                    "p (t x) -> p t x", t=4
                                )[:, :nt, 128:256].rearrange(
                                    "p t (h x) -> p t h x", h=H),
                                in1=expw[:, :nt * H].rearrange(
                                    "p (t h) -> p t h", h=H)
                                .to_broadcast([128, nt, H, D]),
                                op=ALU.mult)
                            nc.vector.tensor_copy(
                                out=rv[:, :nt, 128:132],
                                in_=expw[:, :nt * H].rearrange(
                                    "p (t h) -> p t h", h=H))
                            # aggregation (132 cols: numerator+denominator)
                            if debug and g == 0 and hf == 0 and rel == "w":
                                nc.sync.dma_start(out=dbg["rhsw"][:],
                                                  in_=rhs_t[:])
                                nc.sync.dma_start(out=dbg["hew"][:], in_=he[:])
                            if debug and g == 0 and hf == 0 and rel == "c":
                                nc.sync.dma_start(out=dbg["he"][:], in_=he[:])
                                nc.sync.dma_start(out=dbg["qesb"][:],
                                                  in_=qe_sb[:])
                                nc.sync.dma_start(out=dbg["prod"][:],
                                                  in_=prod[:])
                                nc.sync.dma_start(out=dbg["alpha"][:],
                                                  in_=alpha[:])
                                nc.sync.dma_start(out=dbg["expw"][:],
                                                  in_=expw[:])
                                nc.sync.dma_start(out=dbg["rhs"][:],
                                                  in_=rhs_t[:])
                            ro = 0 if rel == "c" else 132
                            for j, bl, fi, la in tmeta:
                                nc.tensor.matmul(
                                    out=agg[:, bl * 512 + ro:
                                            bl * 512 + ro + 132],
                                    lhsT=stt[:, j * 256:j * 256 + 128],
                                    rhs=rhs_t[:, j * 132:(j + 1) * 132],
                                    start=fi, stop=la)
                        # ---- normalize half ----
                        agv = agg[:].rearrange("p (b x) -> p b x", b=2)
                        dsb = sb.tile([128, 16], F32, tag="dsb")
                        nc.vector.tensor_scalar(
                            out=dsb[:].rearrange("p (b r h) -> p b r h",
                                                 b=2, r=2),
                            in0=agv[:, :, 0:264].rearrange(
                                "p b (r y) -> p b r y", r=2)[:, :, :, 128:132],
                            scalar1=1e-6, scalar2=None, op0=ALU.add)
                        rcp = sb.tile([128, 16], F32, tag="rcp")
                        nc.vector.reciprocal(out=rcp[:], in_=dsb[:])
                        tmpc = sb.tile([128, 256], BF16, tag="tmpc")
                        tmpw = sb.tile([128, 256], BF16, tag="tmpw")
                        for ri, tmp in ((0, tmpc), (1, tmpw)):
                            nc.vector.tensor_tensor(
                                out=tmp[:].rearrange(
                                    "p (b h x) -> p b h x", b=2, h=H),
                                in0=agv[:, :, ri * 132:ri * 132 + 128]
                                .rearrange("p b (h x) -> p b h x", h=H),
                                in1=rcp[:].rearrange(
                                    "p (b r h) -> p b r h", b=2, r=2
                                )[:, :, ri, :].to_broadcast([128, 2, H, D]),
                                op=ALU.mult)
                        if debug and g == 0 and hf == 0:
                            agcp = sb.tile([128, 528], F32, tag="agcp")
                            nc.vector.tensor_copy(
                                out=agcp[:].rearrange(
                                    "p (b y) -> p b y", b=2),
                                in_=agv[:, :, 0:264])
                            nc.sync.dma_start(out=dbg["aggd"][:], in_=agcp[:])
                            nc.sync.dma_start(out=dbg["rcpd"][:], in_=rcp[:])
                            nc.sync.dma_start(out=dbg["tmpc"][:], in_=tmpc[:])
                            nc.sync.dma_start(out=dbg["tmpw"][:], in_=tmpw[:])
                        nc.gpsimd.tensor_tensor(
                            out=op_gt[g][:, hf * 256:(hf + 1) * 256],
                            in0=tmpc[:], in1=tmpw[:], op=ALU.add)

            tc_.strict_bb_all_engine_barrier()
            if debug:
                for g in range(NG):
                    nc.sync.dma_start(
                        out=dbg["hs"][:, g * 512:(g + 1) * 512],
                        in_=hs_gt[g][:])
                    nc.sync.dma_start(
                        out=dbg["op"][:, g * 512:(g + 1) * 512],
                        in_=op_gt[g][:])

            # ---------------- pass 2 ----------------
            with tc_.tile_pool(name="f_sb", bufs=3) as fsb, \
                 tc_.tile_pool(name="f_tp", bufs=2, space="PSUM") as ftp, \
                 tc_.tile_pool(name="f_ps", bufs=2, space="PSUM") as fps:
                for g in range(NG):
                    tp = ftp.tile([128, BG * 128], BF16, tag="tp")
                    for b in range(BG):
                        nc.tensor.transpose(
                            out=tp[:, b * 128:(b + 1) * 128],
                            in_=op_gt[g][:, b * 128:(b + 1) * 128],
                            identity=t_ident[:])
                    gl = fsb.tile([128, BG * 128], BF16, tag="gl")
                    nc.scalar.activation(out=gl[:], in_=tp[:], func=AF.Gelu,
                                         bias=t_bgelu[:, :1], scale=1.0)
                    f_ps = fps.tile([OUT, BG * 128], F32, tag="f")
                    nc.tensor.matmul(out=f_ps[:], lhsT=t_wgh[:, 0:OUT],
                                     rhs=gl[:], start=True, stop=False)
                    nc.tensor.matmul(out=f_ps[:], lhsT=t_wgh[:, OUT:2 * OUT],
                                     rhs=hs_gt[g][:], start=False, stop=True)
                    fo = fsb.tile([OUT, BG * 128], F32, tag="fo")
                    if has_bfin:
                        nc.scalar.activation(out=fo[:], in_=f_ps[:],
                                             func=AF.Identity,
                                             bias=t_bfin[:, :1], scale=1.0)
                    else:
                        nc.scalar.activation(out=fo[:], in_=f_ps[:],
                                             func=AF.Copy)
                    nc.sync.dma_start(
                        out=out_t[:, g * BG * 128:(g + 1) * BG * 128],
                        in_=fo[:])

    nc.compile()
    return nc


# ---------------------------------------------------------------- entry

def kernel(**inputs):
    cfg, in_maps, perms = _prep_all(inputs)
    key = (cfg["NB"], cfg["NTC"], cfg["NTW"], cfg["tc"], cfg["tw"],
           cfg["flags"], cfg["ncp"], cfg["nca"])
    if key not in _BUILD_CACHE:
        _BUILD_CACHE[key] = build_program(cfg)
    nc = _BUILD_CACHE[key]
    res = run_bass_kernel_spmd(nc, in_maps,
                               core_ids=list(range(cfg["n_cores"])))
    sh = cfg["sh"]
    outs = []
    for m in range(cfg["n_cores"]):
        o = np.asarray(res.results[m]["out"])  # [OUT, NB*128]
        perm = perms[m]
        full = np.zeros((sh, OUT), np.float32)
        valid = perm >= 0
        full[perm[valid]] = o.T[valid]
        outs.append(full)
    return np.concatenate(outs, axis=0)
